# revision 6
# baseline (speedup 1.0000x reference)
"""Trainium2 Bass kernel for BatchFeatureDecorr (group-whitening normalization).

Math (matches the reference):
  x1 = regroup(x) as [G=64, M] where row r collects every channel c with
       c % 64 == r ... actually c = q*64 + r; rows are the within-group index.
  mean = mean(x1, axis=1)
  cov  = centered_gram / M + eps*I
  D    = cov^(-1/2) via 10 Newton-Schulz iterations
  out  = (W @ D) @ (x1 - mean) + b

Strategy (8 NeuronCores, data-parallel over batch N):
  - each core gets 8 of the 64 batches as 16 tiles of [128 chans, 3136 hw] f32
  - pass 1: per tile, accumulate per-partition sums (DVE) and the 128x128
    raw gram via PE (cast to fp16, PE-transpose 128-col chunks, PE matmul
    accumulating into one PSUM bank)
  - fold 128->64 stats, AllReduce a [64,65] stat block across the 8 cores
  - replicated: cov = G/M - mean mean^T + eps I, Newton-Schulz in fp32,
    Wp^T = D @ W^T, v = b - Wp @ mean
  - pass 2: out = blockdiag(Wp,Wp) @ x + v via fp32r matmuls (full-rate fp32),
    bias-add fused into the PSUM->SBUF copy on the scalar engine
"""

import numpy as np

import concourse.bass as bass
import concourse.bacc as bacc
import concourse.mybir as mybir
import concourse.tile as tile
from concourse import bass_utils

G = 64
EPS = 1e-5
N_ITER = 10
N_CORES = 8

# Full-problem geometry (hardcoded; harness calls kernel() with x (64,256,56,56))
FULL_N = 64
FULL_C = 256
FULL_HW = 56 * 56            # 3136
TILES_PER_CORE = (FULL_N // N_CORES) * (FULL_C // 128)   # 8 * 2 = 16
M_TOTAL = FULL_N * (FULL_C // G) * FULL_HW               # 802816

f32 = mybir.dt.float32
f32r = mybir.dt.float32r
f16 = mybir.dt.float16


def build_program(n_tiles=TILES_PER_CORE, hw=FULL_HW, m_total=M_TOTAL,
                  n_cores=N_CORES):
    nc = bacc.Bacc("TRN2", target_bir_lowering=False, debug=False,
                   num_devices=n_cores)
    xs = nc.dram_tensor("xs", [n_tiles, 128, hw], f32, kind="ExternalInput").ap()
    w1 = nc.dram_tensor("w1", [G, G], f32, kind="ExternalInput").ap()
    b1 = nc.dram_tensor("b1", [G, 1], f32, kind="ExternalInput").ap()
    eye128h = nc.dram_tensor("eye128h", [128, 128], f16, kind="ExternalInput").ap()
    eye64f = nc.dram_tensor("eye64f", [G, G], f32, kind="ExternalInput").ap()
    ones64 = nc.dram_tensor("ones64", [G, G], f32, kind="ExternalInput").ap()
    out = nc.dram_tensor("out", [n_tiles, 128, hw], f32, kind="ExternalOutput").ap()

    with tile.TileContext(nc) as tc:
        _body(tc, xs, w1, b1, eye128h, eye64f, ones64, out,
              n_tiles, hw, m_total, n_cores)
    nc.compile()
    return nc


def _body(tc, xs, w1, b1, eye128h, eye64f, ones64, out,
          n_tiles, hw, m_total, n_cores):
    nc = tc.nc
    AF = mybir.ActivationFunctionType
    n_ch = (hw + 127) // 128          # transpose chunks per tile
    assert hw % 4 == 0

    with tc.tile_pool(name="consts", bufs=1) as consts:
        eye_h = consts.tile([128, 128], f16)
        nc.sync.dma_start(eye_h[:], eye128h)
        eye_f = consts.tile([G, G], f32)
        nc.sync.dma_start(eye_f[:], eye64f)
        ones_sb = consts.tile([G, G], f32)
        nc.sync.dma_start(ones_sb[:], ones64)
        w1_sb = consts.tile([G, G], f32)
        nc.sync.dma_start(w1_sb[:], w1)
        b1_sb = consts.tile([G, 1], f32)
        nc.sync.dma_start(b1_sb[:], b1)

        ssums = consts.tile([128, n_tiles], f32)
        stat_sb = consts.tile([G, 1 + G], f32)
        stot = consts.tile([G, 1 + G], f32)

        # ---------------- pass 1: sums + raw gram ----------------
        with tc.tile_pool(name="covp", bufs=1, space="PSUM") as covp:
            cov_ps = covp.tile([128, 128], f32)
            with (
                tc.tile_pool(name="xt", bufs=3) as xt_pool,
                tc.tile_pool(name="xh", bufs=2) as xh_pool,
                tc.tile_pool(name="tp", bufs=3, space="PSUM") as tp_pool,
                tc.tile_pool(name="xT", bufs=4) as xT_pool,
            ):
                first = True
                for t in range(n_tiles):
                    xt = xt_pool.tile([128, hw], f32, name=f"xt{t}", tag="xt")
                    nc.sync.dma_start(xt[:], xs[t])
                    nc.vector.reduce_sum(ssums[:, t:t + 1], xt[:],
                                         axis=mybir.AxisListType.X)
                    xh = xh_pool.tile([128, hw], f16, name=f"xh{t}", tag="xh")
                    nc.scalar.copy(xh[:], xt[:])
                    for j in range(n_ch):
                        c0 = j * 128
                        cw = min(128, hw - c0)
                        tp = tp_pool.tile([128, 128], f16,
                                          name=f"tp{t}_{j}", tag="tp")
                        nc.tensor.transpose(tp[:cw, :], xh[:, c0:c0 + cw],
                                            eye_h[:])
                        xT = xT_pool.tile([128, 128], f16,
                                          name=f"xT{t}_{j}", tag="xT")
                        if j % 2 == 0:
                            nc.vector.tensor_copy(xT[:cw, :], tp[:cw, :])
                        else:
                            nc.scalar.copy(xT[:cw, :], tp[:cw, :])
                        last = (t == n_tiles - 1 and j == n_ch - 1)
                        nc.tensor.matmul(cov_ps[:], xT[:cw, :], xT[:cw, :],
                                         start=first, stop=last)
                        first = False

            # fold 128 -> 64 (two group-halves share the same r index).
            # HW forbids vector ops whose SBUF inputs sit on different base
            # partitions, so shift the upper half down via SBUF->SBUF DMA.
            ssum_col = consts.tile([128, 1], f32)
            nc.vector.reduce_sum(ssum_col[:], ssums[:],
                                 axis=mybir.AxisListType.X)
            cov128 = consts.tile([128, 128], f32)
            nc.vector.tensor_copy(cov128[:], cov_ps[:])
            shifted = consts.tile([G, 1 + G], f32)
            nc.sync.dma_start(shifted[:, 0:1], ssum_col[G:128, :])
            nc.sync.dma_start(shifted[:, 1:1 + G], cov128[G:128, G:128])
            nc.vector.tensor_add(stat_sb[:, 0:1], ssum_col[0:G, :],
                                 shifted[:, 0:1])
            nc.vector.tensor_add(stat_sb[:, 1:1 + G], cov128[0:G, 0:G],
                                 shifted[:, 1:1 + G])

        # ---------------- all-reduce the [64, 65] stat block ----------------
        with tc.tile_pool(name="dram", bufs=1, space="DRAM") as dram:
            cc_in = dram.tile([G, 1 + G], f32)
            cc_out = dram.tile([G, 1 + G], f32)
            nc.sync.dma_start(cc_in[:], stat_sb[:])
            nc.gpsimd.collective_compute(
                "AllReduce",
                mybir.AluOpType.add,
                replica_groups=[list(range(n_cores))],
                ins=[cc_in[:]],
                outs=[cc_out[:]],
            )
            nc.sync.dma_start(stot[:], cc_out[:])

        # ---------------- replicated stats + Newton-Schulz ----------------
        with (
            tc.tile_pool(name="sm", bufs=1) as sm,
            tc.tile_pool(name="smp", bufs=3, space="PSUM") as smp,
        ):
            inv_m = 1.0 / float(m_total)
            mean = sm.tile([G, 1], f32)
            nc.vector.tensor_scalar_mul(mean[:], stot[:, 0:1], inv_m)

            # meanT = mean^T @ I : [1, 64]
            ps_meanT = smp.tile([1, G], f32, name="ps_meanT", tag="nsp")
            nc.tensor.matmul(ps_meanT[:], mean[:], eye_f[:], start=True,
                             stop=True)
            meanT = sm.tile([1, G], f32)
            nc.vector.tensor_copy(meanT[:], ps_meanT[:])
            # outer = mean mean^T
            ps_outer = smp.tile([G, G], f32, name="ps_outer", tag="nsp")
            nc.tensor.matmul(ps_outer[:], meanT[:], meanT[:], start=True,
                             stop=True)

            cov_sb = sm.tile([G, G], f32)
            nc.vector.tensor_scalar_mul(cov_sb[:], stot[:, 1:1 + G], inv_m)
            nc.vector.tensor_sub(cov_sb[:], cov_sb[:], ps_outer[:])
            eye_eps = sm.tile([G, G], f32)
            nc.vector.tensor_scalar_mul(eye_eps[:], eye_f[:], EPS)
            nc.vector.tensor_add(cov_sb[:], cov_sb[:], eye_eps[:])

            # Frobenius norm, broadcast to all partitions via ones @ q
            sq = sm.tile([G, G], f32)
            nc.vector.tensor_mul(sq[:], cov_sb[:], cov_sb[:])
            q = sm.tile([G, 1], f32)
            nc.vector.reduce_sum(q[:], sq[:], axis=mybir.AxisListType.X)
            ps_tot = smp.tile([G, 1], f32, name="ps_tot", tag="nsp")
            nc.tensor.matmul(ps_tot[:], ones_sb[:], q[:], start=True, stop=True)
            norm = sm.tile([G, 1], f32)
            nc.scalar.sqrt(norm[:], ps_tot[:])
            rnorm = sm.tile([G, 1], f32)
            nc.vector.reciprocal(rnorm[:], norm[:])

            eye15 = sm.tile([G, G], f32)
            nc.vector.tensor_scalar_mul(eye15[:], eye_f[:], 1.5)

            Y = sm.tile([G, G], f32, name="Y0", tag="Ybuf", bufs=2)
            nc.vector.tensor_scalar_mul(Y[:], cov_sb[:], rnorm[:])
            Z = sm.tile([G, G], f32, name="Z0", tag="Zbuf", bufs=2)
            nc.vector.tensor_copy(Z[:], eye_f[:])

            # All iterates are symmetric polynomials of cov, so A@B is
            # emitted as matmul(lhsT=A, rhs=B) without explicit transposes.
            for it in range(N_ITER):
                psZY = smp.tile([G, G], f32, name=f"psZY{it}", tag="nsp")
                nc.tensor.matmul(psZY[:], Z[:], Y[:], start=True, stop=True)
                T = sm.tile([G, G], f32, name=f"T{it}", tag="Tbuf", bufs=2)
                nc.vector.tensor_scalar(T[:], psZY[:], -0.5, None,
                                        op0=mybir.AluOpType.mult)
                nc.vector.tensor_add(T[:], T[:], eye15[:])
                psZ = smp.tile([G, G], f32, name=f"psZ{it}", tag="nsp")
                nc.tensor.matmul(psZ[:], T[:], Z[:], start=True, stop=True)
                if it < N_ITER - 1:  # Y is dead after the last iteration
                    psY = smp.tile([G, G], f32, name=f"psY{it}", tag="nsp")
                    nc.tensor.matmul(psY[:], Y[:], T[:], start=True, stop=True)
                    Y = sm.tile([G, G], f32, name=f"Y{it + 1}", tag="Ybuf",
                                bufs=2)
                    nc.vector.tensor_copy(Y[:], psY[:])
                Z = sm.tile([G, G], f32, name=f"Z{it + 1}", tag="Zbuf", bufs=2)
                nc.scalar.copy(Z[:], psZ[:])

            # D = Z / sqrt(norm); WpT = D @ W^T; v = b - Wp @ mean
            snorm = sm.tile([G, 1], f32)
            nc.scalar.sqrt(snorm[:], norm[:])
            rsn = sm.tile([G, 1], f32)
            nc.vector.reciprocal(rsn[:], snorm[:])
            D = sm.tile([G, G], f32)
            nc.vector.tensor_scalar_mul(D[:], Z[:], rsn[:])

            psW = smp.tile([G, G], f32, name="psW", tag="nsp")
            nc.tensor.matmul(psW[:], w1_sb[:], eye_f[:], start=True, stop=True)
            WT = sm.tile([G, G], f32)
            nc.vector.tensor_copy(WT[:], psW[:])
            psWp = smp.tile([G, G], f32, name="psWp", tag="nsp")
            nc.tensor.matmul(psWp[:], D[:], WT[:], start=True, stop=True)
            WpT = sm.tile([G, G], f32)
            nc.vector.tensor_copy(WpT[:], psWp[:])

            psvm = smp.tile([G, 1], f32, name="psvm", tag="nsp")
            nc.tensor.matmul(psvm[:], WpT[:], mean[:], start=True, stop=True)
            v = sm.tile([G, 1], f32)
            nc.vector.tensor_sub(v[:], b1_sb[:], psvm[:])

            Wblk = consts.tile([128, 128], f32)
            nc.vector.memset(Wblk[:], 0.0)
            nc.sync.dma_start(Wblk[0:G, 0:G], WpT[:])
            nc.sync.dma_start(Wblk[G:128, G:128], WpT[:])
            vblk = consts.tile([128, 1], f32)
            nc.sync.dma_start(vblk[0:G, :], v[:])
            nc.sync.dma_start(vblk[G:128, :], v[:])

        # ---------------- pass 2: whiten ----------------
        nwc = 392 if hw % 392 == 0 else hw // 4
        assert hw % nwc == 0 and nwc <= 512
        n_w = hw // nwc
        with (
            tc.tile_pool(name="x2", bufs=3) as x2_pool,
            tc.tile_pool(name="po", bufs=4, space="PSUM") as po_pool,
            tc.tile_pool(name="os", bufs=3) as os_pool,
        ):
            for t in range(n_tiles):
                x2 = x2_pool.tile([128, hw], f32, name=f"x2_{t}", tag="x2")
                nc.sync.dma_start(x2[:], xs[t])
                os_t = os_pool.tile([128, hw], f32, name=f"os{t}", tag="os")
                for j in range(n_w):
                    sl = slice(j * nwc, (j + 1) * nwc)
                    po = po_pool.tile([128, nwc], f32,
                                      name=f"po{t}_{j}", tag="po")
                    nc.tensor.matmul(po[:], Wblk[:], x2[:, sl],
                                     start=True, stop=True)
                    nc.scalar.activation(os_t[:, sl], po[:], AF.Identity,
                                         bias=vblk[:], scale=1.0)
                nc.sync.dma_start(out[t], os_t[:])


# ---------------------------------------------------------------------------
# host side
# ---------------------------------------------------------------------------

_PROGRAM_CACHE = {}


def _get_program(key=(TILES_PER_CORE, FULL_HW, M_TOTAL, N_CORES)):
    if key not in _PROGRAM_CACHE:
        _PROGRAM_CACHE[key] = build_program(*key)
    return _PROGRAM_CACHE[key]


def make_in_maps(x, weight1, bias1, n_cores=N_CORES):
    x = np.asarray(x, dtype=np.float32)
    w = np.ascontiguousarray(np.asarray(weight1, dtype=np.float32))
    b = np.ascontiguousarray(np.asarray(bias1, dtype=np.float32).reshape(G, 1))
    n, c, h, wdim = x.shape
    nb = n // n_cores
    hw = h * wdim
    consts = {
        "w1": w,
        "b1": b,
        "eye128h": np.eye(128, dtype=np.float16),
        "eye64f": np.eye(G, dtype=np.float32),
        "ones64": np.ones((G, G), dtype=np.float32),
    }
    in_maps = []
    for i in range(n_cores):
        shard = x[i * nb:(i + 1) * nb].reshape(nb * (c // 128), 128, hw)
        in_maps.append({"xs": np.ascontiguousarray(shard), **consts})
    return in_maps


def unshard_output(results, n=FULL_N, c=FULL_C, h=56, w=56, n_cores=N_CORES):
    nb = n // n_cores
    out = np.empty((n, c, h, w), dtype=np.float32)
    for i in range(n_cores):
        out[i * nb:(i + 1) * nb] = results[i]["out"].reshape(nb, c, h, w)
    return out


def kernel(x, weight1, bias1):
    nc = _get_program()
    in_maps = make_in_maps(x, weight1, bias1)
    res = bass_utils.run_bass_kernel_spmd(nc, in_maps,
                                          core_ids=list(range(N_CORES)))
    return unshard_output(res.results)


if __name__ == "__main__":
    xs = np.random.randn(FULL_N, FULL_C, 56, 56).astype(np.float32)
    w = np.eye(G, dtype=np.float32)
    b = np.zeros((G, 1), dtype=np.float32)
    o = kernel(xs, w, b)
    print(o.shape, o.dtype)


# revision 11
# speedup vs baseline: 1.5970x; 1.5970x over previous
"""Trainium2 Bass kernel for BatchFeatureDecorr (group-whitening normalization).

Math (matches the reference):
  x1 = regroup(x) as [G=64, M] rows indexed by within-group channel r (c = q*G+r)
  mean = mean(x1, axis=1)
  cov  = centered_gram / M + eps*I
  D    = cov^(-1/2) via 10 Newton-Schulz iterations
  out  = (W @ D) @ (x1 - mean) + b

Strategy (8 NeuronCores, data-parallel over batch N):
  - each core gets 8 batches as 16 tiles of [128 chans, 3136 hw], loaded via
    SWDGE cast-DMA straight into float32r tiles (full-rate fp32 matmuls)
  - pass 1: cast tiles to fp16, PE-transpose 128-col chunks (4 per PSUM tile),
    one strided copy per group into persistent fp16 buffers that carry a
    baked-in ones column; PE accumulates [gram | row-sums] in one PSUM bank
    via rhs = [chunk | ones].  The PE stream is software-pipelined (cov
    matmuls trail the transposes by 2 groups) so it never stalls on copies.
  - fold 128->64 stats, AllReduce a [64,65] stat block across the 8 cores
  - replicated: cov = G/M - mean mean^T + eps I, Newton-Schulz in fp32,
    Wp^T = D @ W^T, v = b - Wp @ mean
  - pass 2: out = blockdiag(Wp,Wp) @ x + v with fp32r matmuls; bias-add fused
    into the PSUM->SBUF copy, alternating between Vector and Scalar engines.
    The last 9 x-tiles stay resident in SBUF from pass 1; only 7 reload.
"""

from collections import deque

import numpy as np

import concourse.bass as bass
import concourse.bacc as bacc
import concourse.mybir as mybir
import concourse.tile as tile
from concourse import bass_utils

G = 64
EPS = 1e-5
N_ITER = 10
N_CORES = 8

FULL_N = 64
FULL_C = 256
FULL_HW = 56 * 56            # 3136
TILES_PER_CORE = (FULL_N // N_CORES) * (FULL_C // 128)   # 16
M_TOTAL = FULL_N * (FULL_C // G) * FULL_HW               # 802816

f32 = mybir.dt.float32
f32r = mybir.dt.float32r
f16 = mybir.dt.float16


def build_program(n_tiles=TILES_PER_CORE, hw=FULL_HW, m_total=M_TOTAL,
                  n_cores=N_CORES, n_resident=9):
    nc = bacc.Bacc("TRN2", target_bir_lowering=False, debug=False,
                   num_devices=n_cores)
    xs = nc.dram_tensor("xs", [n_tiles, 128, hw], f32, kind="ExternalInput").ap()
    w1 = nc.dram_tensor("w1", [G, G], f32, kind="ExternalInput").ap()
    b1 = nc.dram_tensor("b1", [G, 1], f32, kind="ExternalInput").ap()
    eye128h = nc.dram_tensor("eye128h", [128, 128], f16, kind="ExternalInput").ap()
    eye64f = nc.dram_tensor("eye64f", [G, G], f32, kind="ExternalInput").ap()
    ones64 = nc.dram_tensor("ones64", [G, G], f32, kind="ExternalInput").ap()
    out = nc.dram_tensor("out", [n_tiles, 128, hw], f32, kind="ExternalOutput").ap()

    with tile.TileContext(nc) as tc:
        _body(tc, xs, w1, b1, eye128h, eye64f, ones64, out,
              n_tiles, hw, m_total, n_cores, n_resident)
    nc.compile()
    return nc


def _body(tc, xs, w1, b1, eye128h, eye64f, ones64, out,
          n_tiles, hw, m_total, n_cores, n_resident):
    nc = tc.nc
    AF = mybir.ActivationFunctionType
    n_resident = min(n_resident, n_tiles - 1)
    n_stream = n_tiles - n_resident
    assert 0 < n_stream <= n_tiles

    # transpose chunks (start, width), grouped 4 per PSUM tile
    chunks = []
    c0 = 0
    while c0 < hw:
        cw = min(128, hw - c0)
        chunks.append((c0, cw))
        c0 += cw
    groups = [chunks[i:i + 4] for i in range(0, len(chunks), 4)]
    NXT = 4        # persistent fp16 chunk buffers (PE pipeline depth)
    LOOKAHEAD = 2  # groups the cov matmuls trail behind the transposes

    with tc.tile_pool(name="consts", bufs=1) as consts:
        eye_h = consts.tile([128, 128], f16)
        nc.sync.dma_start(eye_h[:], eye128h)
        eye_f = consts.tile([G, G], f32)
        nc.sync.dma_start(eye_f[:], eye64f)
        ones_sb = consts.tile([G, G], f32)
        nc.sync.dma_start(ones_sb[:], ones64)
        w1_sb = consts.tile([G, G], f32)
        nc.sync.dma_start(w1_sb[:], w1)
        b1_sb = consts.tile([G, 1], f32)
        nc.sync.dma_start(b1_sb[:], b1)

        stat_sb = consts.tile([G, 1 + G], f32)
        stot = consts.tile([G, 1 + G], f32)

        # persistent fp16 chunk buffers: 4 chunks of 129 columns each; the
        # 129th column stays 1.0 forever and extends every gram matmul so the
        # row-sums accumulate in PSUM column 128 for free.
        xTb = []
        for i in range(NXT):
            b = consts.tile([128, 4 * 129], f16, name=f"xTb{i}")
            nc.vector.memset(b[:], 1.0)
            xTb.append(b)

        res_tiles = {}

        # ---------------- pass 1: fp16 transposes + [gram | sums] ----------
        with tc.tile_pool(name="covp", bufs=1, space="PSUM") as covp:
            cov_ps = covp.tile([128, 129], f32)
            with (
                tc.tile_pool(name="xt", bufs=3) as xt_pool,
                tc.tile_pool(name="xh", bufs=2) as xh_pool,
                tc.tile_pool(name="tp", bufs=4, space="PSUM") as tp_pool,
            ):
                state = {"first": True, "gi": 0}
                pend = deque()
                n_groups_total = n_tiles * len(groups)

                def emit_cov(job, last):
                    buf, members = job
                    for k, (c0_, cw_) in enumerate(members):
                        is_last = last and k == len(members) - 1
                        nc.tensor.matmul(
                            cov_ps[:],
                            buf[:cw_, k * 129:k * 129 + 128],
                            buf[:cw_, k * 129:k * 129 + 129],
                            start=state["first"], stop=is_last)
                        state["first"] = False

                for t in range(n_tiles):
                    if t >= n_stream:
                        xt = consts.tile([128, hw], f32r, name=f"res{t}",
                                         tag=f"res{t}")
                        res_tiles[t] = xt
                    else:
                        xt = xt_pool.tile([128, hw], f32r, name=f"xt{t}",
                                          tag="xt")
                    nc.gpsimd.dma_start(xt[:], xs[t])  # SWDGE: casts f32->f32r
                    xh = xh_pool.tile([128, hw], f16, name=f"xh{t}", tag="xh")
                    if t % 2 == 0:
                        nc.scalar.copy(xh[:], xt[:])
                    else:
                        nc.vector.tensor_copy(xh[:], xt[:])
                    for group in groups:
                        L = len(group)
                        cw = group[-1][1]  # only the last chunk can be narrow
                        tp = tp_pool.tile([128, 512], f16,
                                          name=f"tp{state['gi']}", tag="tp")
                        for k, (gc0, gcw) in enumerate(group):
                            nc.tensor.transpose(
                                tp[:gcw, k * 128:(k + 1) * 128],
                                xh[:, gc0:gc0 + gcw], eye_h[:])
                        buf = xTb[state["gi"] % NXT]
                        src = tp[:cw, 0:L * 128].rearrange(
                            "p (l c) -> p l c", c=128)
                        dst = buf[:cw, 0:L * 129].rearrange(
                            "p (l c) -> p l c", c=129)[:, :, 0:128]
                        if state["gi"] % 7 in (1, 3, 5):
                            nc.scalar.copy(dst, src)
                        else:
                            nc.vector.tensor_copy(dst, src)
                        pend.append((buf, group))
                        state["gi"] += 1
                        if len(pend) > LOOKAHEAD:
                            emit_cov(pend.popleft(), last=False)
                while pend:
                    emit_cov(pend.popleft(), last=not pend)

            # fold 128 -> 64 (cross-partition moves via SBUF->SBUF DMA)
            cov128 = consts.tile([128, 129], f32)
            nc.vector.tensor_copy(cov128[:], cov_ps[:])
            shifted = consts.tile([G, 1 + G], f32)
            nc.sync.dma_start(shifted[:, 0:1], cov128[G:128, 128:129])
            nc.sync.dma_start(shifted[:, 1:1 + G], cov128[G:128, G:128])
            nc.vector.tensor_add(stat_sb[:, 0:1], cov128[0:G, 128:129],
                                 shifted[:, 0:1])
            nc.vector.tensor_add(stat_sb[:, 1:1 + G], cov128[0:G, 0:G],
                                 shifted[:, 1:1 + G])

        # ---------------- all-reduce the [64, 65] stat block ----------------
        with tc.tile_pool(name="dram", bufs=1, space="DRAM") as dram:
            cc_in = dram.tile([G, 1 + G], f32)
            cc_out = dram.tile([G, 1 + G], f32)
            nc.sync.dma_start(cc_in[:], stat_sb[:])
            nc.gpsimd.collective_compute(
                "AllReduce",
                mybir.AluOpType.add,
                replica_groups=[list(range(n_cores))],
                ins=[cc_in[:]],
                outs=[cc_out[:]],
            )
            nc.sync.dma_start(stot[:], cc_out[:])

        # ---------------- replicated stats + Newton-Schulz ----------------
        with (
            tc.tile_pool(name="sm", bufs=1) as sm,
            tc.tile_pool(name="smp", bufs=3, space="PSUM") as smp,
        ):
            inv_m = 1.0 / float(m_total)
            mean = sm.tile([G, 1], f32)
            nc.vector.tensor_scalar_mul(mean[:], stot[:, 0:1], inv_m)

            ps_meanT = smp.tile([1, G], f32, name="ps_meanT", tag="nsp")
            nc.tensor.matmul(ps_meanT[:], mean[:], eye_f[:], start=True,
                             stop=True)
            meanT = sm.tile([1, G], f32)
            nc.vector.tensor_copy(meanT[:], ps_meanT[:])
            ps_outer = smp.tile([G, G], f32, name="ps_outer", tag="nsp")
            nc.tensor.matmul(ps_outer[:], meanT[:], meanT[:], start=True,
                             stop=True)

            cov_sb = sm.tile([G, G], f32)
            nc.vector.tensor_scalar_mul(cov_sb[:], stot[:, 1:1 + G], inv_m)
            nc.vector.tensor_sub(cov_sb[:], cov_sb[:], ps_outer[:])
            eye_eps = sm.tile([G, G], f32)
            nc.vector.tensor_scalar_mul(eye_eps[:], eye_f[:], EPS)
            nc.vector.tensor_add(cov_sb[:], cov_sb[:], eye_eps[:])

            sq = sm.tile([G, G], f32)
            nc.vector.tensor_mul(sq[:], cov_sb[:], cov_sb[:])
            q = sm.tile([G, 1], f32)
            nc.vector.reduce_sum(q[:], sq[:], axis=mybir.AxisListType.X)
            ps_tot = smp.tile([G, 1], f32, name="ps_tot", tag="nsp")
            nc.tensor.matmul(ps_tot[:], ones_sb[:], q[:], start=True, stop=True)
            norm = sm.tile([G, 1], f32)
            nc.scalar.sqrt(norm[:], ps_tot[:])
            rnorm = sm.tile([G, 1], f32)
            nc.vector.reciprocal(rnorm[:], norm[:])

            eye15 = sm.tile([G, G], f32)
            nc.vector.tensor_scalar_mul(eye15[:], eye_f[:], 1.5)

            Y = sm.tile([G, G], f32, name="Y0", tag="Ybuf", bufs=2)
            nc.vector.tensor_scalar_mul(Y[:], cov_sb[:], rnorm[:])
            Z = sm.tile([G, G], f32, name="Z0", tag="Zbuf", bufs=2)
            nc.vector.tensor_copy(Z[:], eye_f[:])

            # all iterates are symmetric polynomials of cov: A@B emitted as
            # matmul(lhsT=A, rhs=B) without explicit transposes
            for it in range(N_ITER):
                psZY = smp.tile([G, G], f32, name=f"psZY{it}", tag="nsp")
                nc.tensor.matmul(psZY[:], Z[:], Y[:], start=True, stop=True)
                T = sm.tile([G, G], f32, name=f"T{it}", tag="Tbuf", bufs=2)
                nc.vector.tensor_scalar(T[:], psZY[:], -0.5, None,
                                        op0=mybir.AluOpType.mult)
                nc.vector.tensor_add(T[:], T[:], eye15[:])
                psZ = smp.tile([G, G], f32, name=f"psZ{it}", tag="nsp")
                nc.tensor.matmul(psZ[:], T[:], Z[:], start=True, stop=True)
                if it < N_ITER - 1:  # Y is dead after the last iteration
                    psY = smp.tile([G, G], f32, name=f"psY{it}", tag="nsp")
                    nc.tensor.matmul(psY[:], Y[:], T[:], start=True, stop=True)
                    Y = sm.tile([G, G], f32, name=f"Y{it + 1}", tag="Ybuf",
                                bufs=2)
                    nc.vector.tensor_copy(Y[:], psY[:])
                Z = sm.tile([G, G], f32, name=f"Z{it + 1}", tag="Zbuf", bufs=2)
                nc.scalar.copy(Z[:], psZ[:])

            # D = Z / sqrt(norm); WpT = D @ W^T; v = b - Wp @ mean
            snorm = sm.tile([G, 1], f32)
            nc.scalar.sqrt(snorm[:], norm[:])
            rsn = sm.tile([G, 1], f32)
            nc.vector.reciprocal(rsn[:], snorm[:])
            D = sm.tile([G, G], f32)
            nc.vector.tensor_scalar_mul(D[:], Z[:], rsn[:])

            psW = smp.tile([G, G], f32, name="psW", tag="nsp")
            nc.tensor.matmul(psW[:], w1_sb[:], eye_f[:], start=True, stop=True)
            WT = sm.tile([G, G], f32)
            nc.vector.tensor_copy(WT[:], psW[:])
            psWp = smp.tile([G, G], f32, name="psWp", tag="nsp")
            nc.tensor.matmul(psWp[:], D[:], WT[:], start=True, stop=True)
            WpT = sm.tile([G, G], f32)
            nc.vector.tensor_copy(WpT[:], psWp[:])

            psvm = smp.tile([G, 1], f32, name="psvm", tag="nsp")
            nc.tensor.matmul(psvm[:], WpT[:], mean[:], start=True, stop=True)
            v = sm.tile([G, 1], f32)
            nc.vector.tensor_sub(v[:], b1_sb[:], psvm[:])

            Wblk = consts.tile([128, 128], f32r)
            nc.vector.memset(Wblk[:].bitcast(f32), 0.0)
            nc.gpsimd.dma_start(Wblk[0:G, 0:G], WpT[:])      # SWDGE cast
            nc.gpsimd.dma_start(Wblk[G:128, G:128], WpT[:])  # SWDGE cast
            vblk = consts.tile([128, 1], f32)
            nc.sync.dma_start(vblk[0:G, :], v[:])
            nc.sync.dma_start(vblk[G:128, :], v[:])

        # ---------------- pass 2: whiten ----------------
        nwc = 392 if hw % 392 == 0 else hw // 4
        assert hw % nwc == 0 and 256 <= nwc <= 512 or hw < 3136
        n_w = hw // nwc
        half = hw // 2
        with (
            tc.tile_pool(name="x2", bufs=3) as x2_pool,
            tc.tile_pool(name="po", bufs=4, space="PSUM") as po_pool,
            tc.tile_pool(name="os", bufs=3) as os_pool,
        ):
            for t in range(n_tiles):
                if t in res_tiles:
                    x2 = res_tiles[t]
                else:
                    x2 = x2_pool.tile([128, hw], f32r, name=f"x2_{t}",
                                      tag="xt")
                    nc.gpsimd.dma_start(x2[:], xs[t])  # SWDGE cast f32->f32r
                os_t = os_pool.tile([128, half], f32, name=f"os{t}a", tag="os")
                for j in range(n_w):
                    if j == n_w // 2:
                        nc.sync.dma_start(out[t][:, 0:half], os_t[:])
                        os_t = os_pool.tile([128, half], f32,
                                            name=f"os{t}b", tag="os")
                    sl = slice(j * nwc, (j + 1) * nwc)
                    osl = slice(j * nwc - (half if j >= n_w // 2 else 0),
                                (j + 1) * nwc - (half if j >= n_w // 2 else 0))
                    po = po_pool.tile([128, nwc], f32,
                                      name=f"po{t}_{j}", tag="po")
                    nc.tensor.matmul(po[:], Wblk[:], x2[:, sl],
                                     start=True, stop=True)
                    if (t + j) % 2 == 0:
                        nc.scalar.activation(os_t[:, osl], po[:], AF.Identity,
                                             bias=vblk[:], scale=1.0)
                    else:
                        nc.vector.tensor_scalar_add(os_t[:, osl], po[:],
                                                    vblk[:])
                nc.sync.dma_start(out[t][:, half:hw], os_t[:])


# ---------------------------------------------------------------------------
# host side
# ---------------------------------------------------------------------------

_PROGRAM_CACHE = {}


def _get_program(key=(TILES_PER_CORE, FULL_HW, M_TOTAL, N_CORES)):
    if key not in _PROGRAM_CACHE:
        _PROGRAM_CACHE[key] = build_program(*key)
    return _PROGRAM_CACHE[key]


def make_in_maps(x, weight1, bias1, n_cores=N_CORES):
    x = np.asarray(x, dtype=np.float32)
    w = np.ascontiguousarray(np.asarray(weight1, dtype=np.float32))
    b = np.ascontiguousarray(np.asarray(bias1, dtype=np.float32).reshape(G, 1))
    n, c, h, wdim = x.shape
    nb = n // n_cores
    hw = h * wdim
    consts = {
        "w1": w,
        "b1": b,
        "eye128h": np.eye(128, dtype=np.float16),
        "eye64f": np.eye(G, dtype=np.float32),
        "ones64": np.ones((G, G), dtype=np.float32),
    }
    in_maps = []
    for i in range(n_cores):
        shard = x[i * nb:(i + 1) * nb].reshape(nb * (c // 128), 128, hw)
        in_maps.append({"xs": np.ascontiguousarray(shard), **consts})
    return in_maps


def unshard_output(results, n=FULL_N, c=FULL_C, h=56, w=56, n_cores=N_CORES):
    nb = n // n_cores
    out = np.empty((n, c, h, w), dtype=np.float32)
    for i in range(n_cores):
        out[i * nb:(i + 1) * nb] = results[i]["out"].reshape(nb, c, h, w)
    return out


def kernel(x, weight1, bias1):
    nc = _get_program()
    in_maps = make_in_maps(x, weight1, bias1)
    res = bass_utils.run_bass_kernel_spmd(nc, in_maps,
                                          core_ids=list(range(N_CORES)))
    return unshard_output(res.results)


if __name__ == "__main__":
    xs = np.random.randn(FULL_N, FULL_C, 56, 56).astype(np.float32)
    w = np.eye(G, dtype=np.float32)
    b = np.zeros((G, 1), dtype=np.float32)
    o = kernel(xs, w, b)
    print(o.shape, o.dtype)
